# revision 18
# baseline (speedup 1.0000x reference)
"""Trainium2 Bass kernel for nn_CVX_Reasoning_Engine.

MLP (16384x512 -> 512 -> 256 -> 128 -> 64 -> 256) with LeakyReLU(0.2),
followed by a closed-form per-object/axis QP solve.

Strategy (v2, bf16):
- Pure data parallel over 8 NeuronCores (2048 batch rows each).
- All matmuls in bf16 (1 col/cycle on PE, FWL halves LDWEIGHTS); z is
  shipped bf16 (halves input DMA); output is stored fp16 (halves
  output DMA); QP runs in fp16 on DVE (2x mode).
- Host-side prep: fold `bounds` contribution of the concat into layer-1
  bias; transpose z so activations flow feature-major on-chip; append
  the layer-5 bias as an extra ones-row of the last activation (K=65
  matmul) so layer 5 exits batch-major.
- Elementwise work is spread across engines so none exceeds PE time:
  ACT does Prelu+bias for half the regions; the rest get a Pool
  in-place bias-add on PSUM followed by a single DVE op
  lrelu(v) = max(0.2*v, v).
- QP closed form without branches:
    g0 = max(pg, 1); s0 = max(pp, lo) + g0; w = min(s0, hi)
    u  = min(0.5*(pp - pg + hi), hi-1)
    x  = max(min(max(pp, lo), u), lo)
"""

import numpy as np

BS, Z, NOBJ = 16384, 512, 64
NCORES = 8
BSC = BS // NCORES            # 2048 batch rows per core
P = 128

# wk (bf16) packed offsets, in elements per partition
_W2O, _W3O, _W4O, _W5O = 0, 1024, 1280, 1344
_WKW = 1600

# bisection knobs
CFG = {
    "dve_acts": False,   # offload some lrelu+bias regions to DVE 2-op path
    "qp_f16": False,     # QP in fp16 (else fp32)
}

_cache = {}


def _build(b0, b1, b2, b3, reps=1, chunks=(1024, 1024), hw_loop=0):
    import concourse.tile as tile
    from concourse import bacc, mybir

    f32 = mybir.dt.float32
    bf16 = mybir.dt.bfloat16
    f16 = mybir.dt.float16
    AF = mybir.ActivationFunctionType
    Alu = mybir.AluOpType

    nc = bacc.Bacc("TRN2", target_bir_lowering=False, debug=False,
                   num_devices=NCORES)

    zt_d = nc.dram_tensor("zt", (Z, BSC), bf16, kind="ExternalInput").ap()
    w1_d = nc.dram_tensor("w1", (512, 512), bf16, kind="ExternalInput").ap()
    wk_d = nc.dram_tensor("wk", (P, _WKW), bf16, kind="ExternalInput").ap()
    bia_d = nc.dram_tensor("bia", (P, 8), f32, kind="ExternalInput").ap()
    o_d = nc.dram_tensor("o", (BSC, 256), f16, kind="ExternalOutput").ap()

    lo_x, hi_x = float(b0), float(b2)
    lo_y, hi_y = float(b1), float(b3)

    with tile.TileContext(nc) as tc:
        with (
            tc.tile_pool(name="wp", bufs=1) as wp,
            tc.tile_pool(name="zp", bufs=2) as zp,
            tc.tile_pool(name="hp", bufs=2) as hp,
            tc.tile_pool(name="stg", bufs=3) as stg,
            tc.tile_pool(name="scp", bufs=2) as scp,
            tc.tile_pool(name="tmp", bufs=2) as tmp,
            tc.tile_pool(name="big", bufs=3, space="PSUM") as big,
            tc.tile_pool(name="ps5", bufs=2, space="PSUM") as ps5p,
        ):
            # ---- resident weights ----
            w1_sb = wp.tile([P, 4 * 512], bf16, tag="w1")
            w1v = w1_d.rearrange("(k p) m -> p k m", p=P)
            wk_sb = wp.tile([P, _WKW], bf16, tag="wk")
            bia_sb = wp.tile([P, 8], f32, tag="bia")

            def w1k(k):
                return w1_sb[:, k * 512:(k + 1) * 512]

            w2v = wk_sb[:, _W2O:_W2O + 1024]
            w3v = wk_sb[:, _W3O:_W3O + 256]
            w4v = wk_sb[:, _W4O:_W4O + 64]
            w5v = wk_sb[:, _W5O:_W5O + 256]

            def bia(c):
                return bia_sb[:, c:c + 1]

            def load_weights():
                for k in range(4):
                    nc.sync.dma_start(w1_sb[:, k * 512:(k + 1) * 512],
                                      w1v[:, k, :])
                nc.sync.dma_start(wk_sb[:], wk_d)
                nc.sync.dma_start(bia_sb[:], bia_d)

            def rep_body(first_rep):
              col0 = 0
              for ci, W in enumerate(chunks):
                first = (first_rep and ci == 0)
                hfs = []
                off = 0
                while off < W:
                    hw = min(512, W - off)
                    hfs.append((off, hw))
                    off += hw

                # ---- load z chunk (feature-major, per-k split on chunk 0) ----
                zt_n = zp.tile([P, 4 * W], bf16, tag="zt")
                if first:
                    for k in range(4):
                        nc.sync.dma_start(w1_sb[:, k * 512:(k + 1) * 512],
                                          w1v[:, k, :])
                        nc.sync.dma_start(
                            zt_n[:, k * W:(k + 1) * W],
                            zt_d[k * P:(k + 1) * P, col0:col0 + W])
                    nc.sync.dma_start(wk_sb[:], wk_d)
                    nc.sync.dma_start(bia_sb[:], bia_d)
                else:
                    nc.sync.dma_start(
                        zt_n[:].rearrange("p (k c) -> p k c", k=4),
                        zt_d[:, col0:col0 + W]
                            .rearrange("(k p) c -> p k c", p=P))

                # helper: write h = lrelu(psum + bias) into dst.
                # mode "act": one ACT op (Prelu with bias).
                # mode "dve": DVE adds bias PSUM -> SBUF f32 scratch, then
                #             one DVE op lrelu(v) = max(0.2*v, v). (GPSIMD
                #             cannot read PSUM nor run TensorScalarPtr/STT.)
                def act_or_dve(dst, pst_v, b_ap, mode, W=W):
                    if not CFG["dve_acts"]:
                        mode = "act"
                    if mode == "act":
                        nc.scalar.activation(dst, pst_v, AF.Prelu,
                                             bias=b_ap, alpha=0.2)
                    else:
                        sc = scp.tile([P, W], f32, tag="sc")
                        nc.vector.tensor_scalar(sc[:], pst_v, b_ap, None,
                                                Alu.add)
                        nc.vector.scalar_tensor_tensor(
                            dst, sc[:], 0.2, sc[:], Alu.mult, Alu.max)

                # ---- L1: 512 -> 512 ----
                h1_n = hp.tile([P, 4 * W], bf16, tag="h1")
                for m in range(4):
                    pst = big.tile([P, W], f32, tag="big")
                    for k in range(4):
                        for off, hw in hfs:
                            nc.tensor.matmul(
                                pst[:, off:off + hw],
                                w1k(k)[:, m * 128:(m + 1) * 128],
                                zt_n[:, k * W + off:k * W + off + hw],
                                start=(k == 0), stop=(k == 3))
                    act_or_dve(h1_n[:, m * W:(m + 1) * W], pst[:, 0:W],
                               bia(m), "act" if m < 3 else "dve")

                # ---- L2: 512 -> 256 ----
                h2_n = hp.tile([P, 2 * W], bf16, tag="h2")
                for m in range(2):
                    pst = big.tile([P, W], f32, tag="big")
                    for k in range(4):
                        for off, hw in hfs:
                            nc.tensor.matmul(
                                pst[:, off:off + hw],
                                w2v[:, k * 256 + m * 128:k * 256 + (m + 1) * 128],
                                h1_n[:, k * W + off:k * W + off + hw],
                                start=(k == 0), stop=(k == 3))
                    act_or_dve(h2_n[:, m * W:(m + 1) * W], pst[:, 0:W],
                               bia(4 + m), "act" if m == 0 else "dve")

                # ---- L3: 256 -> 128 ----
                h3_n = hp.tile([P, W], bf16, tag="h3")
                pst = big.tile([P, W], f32, tag="big")
                for k in range(2):
                    for off, hw in hfs:
                        nc.tensor.matmul(
                            pst[:, off:off + hw],
                            w3v[:, k * 128:(k + 1) * 128],
                            h2_n[:, k * W + off:k * W + off + hw],
                            start=(k == 0), stop=(k == 1))
                nc.scalar.activation(h3_n[:], pst[:, 0:W], AF.Prelu,
                                     bias=bia(6), alpha=0.2)

                # ---- L4: 128 -> 64 (plus ones row for L5 bias) ----
                h4_n = hp.tile([65, W], bf16, tag="h4")
                pst = big.tile([P, W], f32, tag="big")
                for off, hw in hfs:
                    nc.tensor.matmul(pst[0:64, off:off + hw],
                                     w4v[:], h3_n[:, off:off + hw],
                                     start=True, stop=True)
                nc.scalar.activation(h4_n[0:64, :], pst[0:64, 0:W],
                                     AF.Prelu, bias=bia(7)[0:64], alpha=0.2)
                nc.gpsimd.memset(h4_n[64:65, :], 1.0)

                # ---- L5 + QP per double-staging (512 batch rows) ----
                nds = W // 512
                for ds in range(nds):
                    fq = f16 if CFG["qp_f16"] else f32
                    p_sb = stg.tile([P, 1024], fq, tag="p")
                    o_sb = stg.tile([P, 1024], f16, tag="o")
                    for st2 in range(2):
                        p5 = ps5p.tile([P, 512], f32, tag="l5")
                        for j in range(2):
                            sub = ds * 4 + st2 * 2 + j
                            nc.tensor.matmul(
                                p5[:, j * 256:(j + 1) * 256],
                                h4_n[0:65, sub * P:(sub + 1) * P],
                                w5v[0:65, :], start=True, stop=True)
                        dst = p_sb[:, st2 * 512:(st2 + 1) * 512]
                        nc.vector.tensor_copy(dst, p5[:])

                    # QP solve (batch-major, fp16, immediates baked)
                    S = 4
                    pv = p_sb[:].rearrange("p (s o c) -> p s o c", s=S, o=NOBJ)
                    ov = o_sb[:].rearrange("p (s o c) -> p s o c", s=S, o=NOBJ)
                    if b0 == b1 and b2 == b3:
                        groups = [((0, 2), 2, lo_x, hi_x)]
                    else:
                        groups = [((0, 2), 1, lo_x, hi_x),
                                  ((1, 3), 1, lo_y, hi_y)]
                    for (cpp, cpg), cw, lo, hi in groups:
                        fd = S * NOBJ * cw
                        pp = pv[:, :, :, cpp:cpp + cw]
                        pg = pv[:, :, :, cpg:cpg + cw]

                        def tv(t, fd=fd, cw=cw):
                            return t[:, 0:fd].rearrange(
                                "p (s o c) -> p s o c", s=S, o=NOBJ)

                        g0 = tmp.tile([P, fd], fq, tag="g0")
                        s0 = tmp.tile([P, fd], fq, tag="s0")
                        u = tmp.tile([P, fd], fq, tag="u")
                        t1 = tmp.tile([P, fd], fq, tag="t1")
                        g0v, s0v, uv, t1v = map(tv, (g0, s0, u, t1))
                        # g0 = max(pg, 1); s0 = max(pp, lo) + g0; w = min(s0, hi)
                        nc.gpsimd.tensor_scalar_max(g0v, pg, 1.0)
                        nc.vector.scalar_tensor_tensor(
                            s0v, pp, lo, g0v, Alu.max, Alu.add)
                        nc.gpsimd.tensor_scalar_min(
                            ov[:, :, :, cpg:cpg + cw], s0v, hi)
                        # u = (pp + hi) - pg ; scale+clip
                        nc.vector.scalar_tensor_tensor(
                            uv, pp, hi, pg, Alu.add, Alu.subtract)
                        nc.gpsimd.tensor_scalar(uv, uv, 0.5, hi - 1.0,
                                                Alu.mult, Alu.min)
                        # x = max(min(max(pp, lo), u), lo)
                        nc.vector.scalar_tensor_tensor(
                            t1v, pp, lo, uv, Alu.max, Alu.min)
                        nc.vector.tensor_scalar_max(
                            ov[:, :, :, cpp:cpp + cw], t1v, lo)

                    # ---- store double-staging -> DRAM (contiguous rows) ----
                    r0 = col0 + ds * 512
                    nc.sync.dma_start(
                        o_d[r0:r0 + 512, :].rearrange("(s p) f -> p s f", p=P),
                        o_sb[:].rearrange("p (s f) -> p s f", s=4))
                col0 += W

            if hw_loop:
                # timing mode: weights loaded once, then a device-side loop
                # of `hw_loop` iterations, each running `reps` rep bodies.
                load_weights()
                rep_body(False)
                with tc.For_i(0, hw_loop, 1):
                    for _ in range(reps):
                        rep_body(False)
            else:
                for rep in range(reps):
                    rep_body(rep == 0)

    nc.compile()
    return nc


def _get_nc(b0, b1, b2, b3, reps=1, chunks=(1024, 1024), hw_loop=0):
    key = (b0, b1, b2, b3, reps, tuple(chunks), hw_loop)
    if key not in _cache:
        _cache[key] = _build(b0, b1, b2, b3, reps, chunks, hw_loop)
    return _cache[key]


def _prep_inputs(z, bounds, W1, c1, W2, c2, W3, c3, W4, c4, W5, c5):
    import ml_dtypes
    bft = ml_dtypes.bfloat16

    b = np.asarray(bounds, np.float32)
    W1m = np.ascontiguousarray(W1[:Z], np.float32).astype(bft)
    b1 = (np.asarray(c1, np.float32)
          + b @ np.asarray(W1[Z:], np.float32)).astype(np.float32)

    wk = np.zeros((P, _WKW), bft)
    wk[:, _W2O:_W2O + 1024] = (np.asarray(W2, np.float32)
                               .reshape(4, P, 256).transpose(1, 0, 2)
                               .reshape(P, 1024).astype(bft))
    wk[:, _W3O:_W3O + 256] = (np.asarray(W3, np.float32)
                              .reshape(2, P, 128).transpose(1, 0, 2)
                              .reshape(P, 256).astype(bft))
    wk[:, _W4O:_W4O + 64] = np.asarray(W4, np.float32).astype(bft)
    w5a = np.concatenate(
        [np.asarray(W5, np.float32), np.asarray(c5, np.float32)[None, :]], 0)
    wk[0:65, _W5O:_W5O + 256] = w5a.astype(bft)

    bia = np.zeros((P, 8), np.float32)
    bia[:, 0:4] = b1.reshape(4, P).T
    bia[:, 4:6] = np.asarray(c2, np.float32).reshape(2, P).T
    bia[:, 6] = np.asarray(c3, np.float32)
    bia[0:64, 7] = np.asarray(c4, np.float32)

    zT = np.ascontiguousarray(np.asarray(z, np.float32).T).astype(bft)
    common = {"w1": W1m, "wk": wk, "bia": bia}
    in_maps = []
    for i in range(NCORES):
        m = dict(common)
        m["zt"] = np.ascontiguousarray(zT[:, i * BSC:(i + 1) * BSC])
        in_maps.append(m)
    return in_maps, (float(b[0]), float(b[1]), float(b[2]), float(b[3]))


def kernel(z, bounds, W1, c1, W2, c2, W3, c3, W4, c4, W5, c5):
    from concourse.bass_utils import run_bass_kernel_spmd

    in_maps, bvals = _prep_inputs(z, bounds, W1, c1, W2, c2, W3, c3,
                                  W4, c4, W5, c5)
    nc = _get_nc(*bvals)
    res = run_bass_kernel_spmd(nc, in_maps, core_ids=list(range(NCORES)))
    out = np.concatenate(
        [np.asarray(r["o"], np.float32) for r in res.results], axis=0)
    return out.reshape(BS, NOBJ, 4)


# revision 27
# speedup vs baseline: 1.0181x; 1.0181x over previous
"""Trainium2 Bass kernel for nn_CVX_Reasoning_Engine.

MLP (16384x512 -> 512 -> 256 -> 128 -> 64 -> 256) with LeakyReLU(0.2),
followed by a closed-form per-object/axis QP solve.

Strategy (v2, bf16):
- Pure data parallel over 8 NeuronCores (2048 batch rows each).
- All matmuls in bf16 (1 col/cycle on PE, FWL halves LDWEIGHTS); z is
  shipped bf16 (halves input DMA); output is stored fp16 (halves
  output DMA); QP runs in fp16 on DVE (2x mode).
- Host-side prep: fold `bounds` contribution of the concat into layer-1
  bias; transpose z so activations flow feature-major on-chip; append
  the layer-5 bias as an extra ones-row of the last activation (K=65
  matmul) so layer 5 exits batch-major.
- Elementwise work is spread across engines so none exceeds PE time:
  ACT does Prelu+bias for half the regions; the rest get a Pool
  in-place bias-add on PSUM followed by a single DVE op
  lrelu(v) = max(0.2*v, v).
- QP closed form without branches:
    g0 = max(pg, 1); s0 = max(pp, lo) + g0; w = min(s0, hi)
    u  = min(0.5*(pp - pg + hi), hi-1)
    x  = max(min(max(pp, lo), u), lo)
"""

import numpy as np

BS, Z, NOBJ = 16384, 512, 64
NCORES = 8
BSC = BS // NCORES            # 2048 batch rows per core
P = 128

# wk (bf16) packed offsets, in elements per partition
_W2O, _W3O, _W4O, _W5O = 0, 1024, 1280, 1344
_WKW = 1600

# bisection knobs
CFG = {
    "dve_acts": False,   # offload some lrelu+bias regions to DVE 2-op path
    "qp_f16": True,     # QP in fp16 (else fp32)
    "ablate": set(),     # subset of {"zload","acts","l5copy","qp","store"}
}

_cache = {}


def _build(b0, b1, b2, b3, reps=1, chunks=(1024, 1024), hw_loop=0):
    import concourse.tile as tile
    from concourse import bacc, mybir

    f32 = mybir.dt.float32
    bf16 = mybir.dt.bfloat16
    f16 = mybir.dt.float16
    AF = mybir.ActivationFunctionType
    Alu = mybir.AluOpType

    nc = bacc.Bacc("TRN2", target_bir_lowering=False, debug=False,
                   num_devices=NCORES)

    zt_d = nc.dram_tensor("zt", (Z, BSC), bf16, kind="ExternalInput").ap()
    w1_d = nc.dram_tensor("w1", (512, 512), bf16, kind="ExternalInput").ap()
    wk_d = nc.dram_tensor("wk", (P, _WKW), bf16, kind="ExternalInput").ap()
    bia_d = nc.dram_tensor("bia", (P, 8), f32, kind="ExternalInput").ap()
    o_d = nc.dram_tensor("o", (BSC, 256), f16, kind="ExternalOutput").ap()

    lo_x, hi_x = float(b0), float(b2)
    lo_y, hi_y = float(b1), float(b3)

    with tile.TileContext(nc) as tc:
        with (
            tc.tile_pool(name="wp", bufs=1) as wp,
            tc.tile_pool(name="zp", bufs=2) as zp,
            tc.tile_pool(name="hp", bufs=2) as hp,
            tc.tile_pool(name="stg", bufs=3) as stg,
            tc.tile_pool(name="scp", bufs=2) as scp,
            tc.tile_pool(name="tmp", bufs=2) as tmp,
            tc.tile_pool(name="big", bufs=3, space="PSUM") as big,
            tc.tile_pool(name="ps5", bufs=2, space="PSUM") as ps5p,
        ):
            # ---- resident weights ----
            w1_sb = wp.tile([P, 4 * 512], bf16, tag="w1")
            w1v = w1_d.rearrange("(k p) m -> p k m", p=P)
            wk_sb = wp.tile([P, _WKW], bf16, tag="wk")
            bia_sb = wp.tile([P, 8], f32, tag="bia")

            def w1k(k):
                return w1_sb[:, k * 512:(k + 1) * 512]

            w2v = wk_sb[:, _W2O:_W2O + 1024]
            w3v = wk_sb[:, _W3O:_W3O + 256]
            w4v = wk_sb[:, _W4O:_W4O + 64]
            w5v = wk_sb[:, _W5O:_W5O + 256]

            def bia(c):
                return bia_sb[:, c:c + 1]

            def load_weights():
                for k in range(4):
                    nc.sync.dma_start(w1_sb[:, k * 512:(k + 1) * 512],
                                      w1v[:, k, :])
                nc.sync.dma_start(wk_sb[:], wk_d)
                nc.sync.dma_start(bia_sb[:], bia_d)

            def rep_body(first_rep):
              col0 = 0
              for ci, W in enumerate(chunks):
                first = (first_rep and ci == 0)
                hfs = []
                off = 0
                while off < W:
                    hw = min(512, W - off)
                    hfs.append((off, hw))
                    off += hw

                # ---- load z chunk (feature-major, per-k split on chunk 0) ----
                zt_n = zp.tile([P, 4 * W], bf16, tag="zt")
                if "zload" in CFG["ablate"]:
                    nc.gpsimd.memset(zt_n[0:1, 0:2], 0.0)
                elif first:
                    for k in range(4):
                        nc.sync.dma_start(w1_sb[:, k * 512:(k + 1) * 512],
                                          w1v[:, k, :])
                        nc.sync.dma_start(
                            zt_n[:, k * W:(k + 1) * W],
                            zt_d[k * P:(k + 1) * P, col0:col0 + W])
                    nc.sync.dma_start(wk_sb[:], wk_d)
                    nc.sync.dma_start(bia_sb[:], bia_d)
                else:
                    nc.sync.dma_start(
                        zt_n[:].rearrange("p (k c) -> p k c", k=4),
                        zt_d[:, col0:col0 + W]
                            .rearrange("(k p) c -> p k c", p=P))

                # helper: write h = lrelu(psum + bias) into dst.
                # mode "act": one ACT op (Prelu with bias).
                # mode "dve": DVE adds bias PSUM -> SBUF f32 scratch, then
                #             one DVE op lrelu(v) = max(0.2*v, v). (GPSIMD
                #             cannot read PSUM nor run TensorScalarPtr/STT.)
                def act_or_dve(dst, pst_v, b_ap, mode, W=W):
                    if "acts" in CFG["ablate"]:
                        return
                    if not CFG["dve_acts"]:
                        mode = "act"
                    if mode == "act":
                        nc.scalar.activation(dst, pst_v, AF.Prelu,
                                             bias=b_ap, alpha=0.2)
                    else:
                        sc = scp.tile([P, W], f32, tag="sc")
                        nc.vector.tensor_scalar(sc[:], pst_v, b_ap, None,
                                                Alu.add)
                        nc.vector.scalar_tensor_tensor(
                            dst, sc[:], 0.2, sc[:], Alu.mult, Alu.max)

                abl = CFG["ablate"]
                # ---- L1: 512 -> 512 ----
                h1_n = zt_n if "acts" in abl else hp.tile(
                    [P, 4 * W], bf16, tag="h1")
                for m in range(4):
                    pst = big.tile([P, W], f32, tag="big")
                    for k in range(4):
                        for off, hw in hfs:
                            nc.tensor.matmul(
                                pst[:, off:off + hw],
                                w1k(k)[:, m * 128:(m + 1) * 128],
                                zt_n[:, k * W + off:k * W + off + hw],
                                start=(k == 0), stop=(k == 3))
                    act_or_dve(h1_n[:, m * W:(m + 1) * W], pst[:, 0:W],
                               bia(m), "act" if m < 3 else "dve")

                # ---- L2: 512 -> 256 ----
                h2_n = zt_n if "acts" in abl else hp.tile(
                    [P, 2 * W], bf16, tag="h2")
                for m in range(2):
                    pst = big.tile([P, W], f32, tag="big")
                    for k in range(4):
                        for off, hw in hfs:
                            nc.tensor.matmul(
                                pst[:, off:off + hw],
                                w2v[:, k * 256 + m * 128:k * 256 + (m + 1) * 128],
                                h1_n[:, k * W + off:k * W + off + hw],
                                start=(k == 0), stop=(k == 3))
                    act_or_dve(h2_n[:, m * W:(m + 1) * W], pst[:, 0:W],
                               bia(4 + m), "act" if m == 0 else "dve")

                # ---- L3: 256 -> 128 ----
                h3_n = zt_n if "acts" in abl else hp.tile(
                    [P, W], bf16, tag="h3")
                pst = big.tile([P, W], f32, tag="big")
                for k in range(2):
                    for off, hw in hfs:
                        nc.tensor.matmul(
                            pst[:, off:off + hw],
                            w3v[:, k * 128:(k + 1) * 128],
                            h2_n[:, k * W + off:k * W + off + hw],
                            start=(k == 0), stop=(k == 1))
                if "acts" not in CFG["ablate"]:
                    nc.scalar.activation(h3_n[:], pst[:, 0:W], AF.Prelu,
                                         bias=bia(6), alpha=0.2)

                # ---- L4: 128 -> 64 (plus ones row for L5 bias) ----
                h4_n = zt_n if "acts" in abl else hp.tile(
                    [65, W], bf16, tag="h4")
                pst = big.tile([P, W], f32, tag="big")
                for off, hw in hfs:
                    nc.tensor.matmul(pst[0:64, off:off + hw],
                                     w4v[:], h3_n[:, off:off + hw],
                                     start=True, stop=True)
                if "acts" not in CFG["ablate"]:
                    nc.scalar.activation(h4_n[0:64, :], pst[0:64, 0:W],
                                         AF.Prelu, bias=bia(7)[0:64],
                                         alpha=0.2)
                    nc.gpsimd.memset(h4_n[64:65, :], 1.0)

                # ---- L5 + QP per double-staging (512 batch rows) ----
                nds = W // 512
                for ds in range(nds):
                    fq = f16 if CFG["qp_f16"] else f32
                    need_p = "l5copy" not in abl or "qp" not in abl
                    need_o = "qp" not in abl or "store" not in abl
                    p_sb = o_sb = None
                    if need_p:
                        p_sb = stg.tile([P, 1024], fq, tag="p")
                    if need_o:
                        o_sb = stg.tile([P, 1024], f16, tag="o")
                    # W5's columns are host-permuted to [g][c2][o] order
                    # (g=0: position params x,y; g=1: size params w,h), so
                    # p_sb laid out [g=2][s=4][c2=2][o=64] gives the QP
                    # fully contiguous pp/pg operand blocks.
                    for st2 in range(2):
                        p5 = ps5p.tile([P, 512], f32, tag="l5")
                        for j in range(2):
                            sub = ds * 4 + st2 * 2 + j
                            nc.tensor.matmul(
                                p5[:, j * 256:(j + 1) * 256],
                                h4_n[0:65, sub * P:(sub + 1) * P],
                                w5v[0:65, :], start=True, stop=True)
                        if "l5copy" not in CFG["ablate"]:
                            dst = (p_sb[:]
                                   .rearrange("p (g s f) -> p g s f",
                                              g=2, s=4)
                                   [:, :, st2 * 2:(st2 + 1) * 2, :])
                            src = p5[:].rearrange(
                                "p (s2 g f) -> p g s2 f", s2=2, g=2)
                            nc.vector.tensor_copy(dst, src)

                    # QP solve (batch-major, fp16, contiguous blocks)
                    S = 4
                    if b0 == b1 and b2 == b3:
                        groups = [((0, 1), 2, lo_x, hi_x)]
                    else:
                        groups = [((0, 0), 1, lo_x, hi_x),
                                  ((1, 1), 1, lo_y, hi_y)]
                    if "qp" in CFG["ablate"]:
                        groups = []
                    for (c2a, c2b), cw, lo, hi in groups:
                        fd = S * NOBJ * cw
                        if cw == 2:
                            pp = p_sb[:, 0:512]
                            pg = p_sb[:, 512:1024]
                            ovv = o_sb[:].rearrange("p (s q) -> p s q", s=S)
                            ox = ovv[:, :, 0:128]
                            ow = ovv[:, :, 128:256]
                        else:
                            pv5 = p_sb[:].rearrange(
                                "p (g s c2 o) -> p g s (c2 o)", g=2, s=4)
                            pp = pv5[:, 0, :, c2a * 64:(c2a + 1) * 64]
                            pg = pv5[:, 1, :, c2a * 64:(c2a + 1) * 64]
                            ovv = o_sb[:].rearrange(
                                "p (s g c2 o) -> p s g (c2 o)", s=4, g=2)
                            ox = ovv[:, :, 0, c2a * 64:(c2a + 1) * 64]
                            ow = ovv[:, :, 1, c2a * 64:(c2a + 1) * 64]

                        g0 = tmp.tile([P, fd], fq, tag="g0")
                        s0 = tmp.tile([P, fd], fq, tag="s0")
                        u = tmp.tile([P, fd], fq, tag="u")
                        t1 = tmp.tile([P, fd], fq, tag="t1")

                        def tv(t, fd=fd):
                            return t[:, 0:fd].rearrange(
                                "p (s q) -> p s q", s=S)

                        mk = (lambda t: t[:]) if cw == 2 else tv

                        # g0 = max(pg, 1); s0 = max(pp, lo) + g0; w = min(s0, hi)
                        nc.gpsimd.tensor_scalar_max(mk(g0), pg, 1.0)
                        nc.vector.scalar_tensor_tensor(
                            mk(s0), pp, lo, mk(g0), Alu.max, Alu.add)
                        nc.gpsimd.tensor_scalar_min(ow, tv(s0), hi)
                        # u = (pp + hi) - pg ; scale+clip
                        nc.vector.scalar_tensor_tensor(
                            mk(u), pp, hi, pg, Alu.add, Alu.subtract)
                        nc.gpsimd.tensor_scalar(mk(u), mk(u), 0.5, hi - 1.0,
                                                Alu.mult, Alu.min)
                        # x = max(min(max(pp, lo), u), lo)
                        nc.vector.scalar_tensor_tensor(
                            mk(t1), pp, lo, mk(u), Alu.max, Alu.min)
                        nc.vector.tensor_scalar_max(ox, tv(t1), lo)

                    # ---- store double-staging -> DRAM (contiguous rows) ----
                    r0 = col0 + ds * 512
                    if "store" not in CFG["ablate"]:
                        o_src = o_sb if "qp" not in abl else p_sb
                        nc.sync.dma_start(
                            o_d[r0:r0 + 512, :]
                                .rearrange("(s p) f -> p s f", p=P),
                            o_src[:].rearrange("p (s f) -> p s f", s=4))
                col0 += W

            if hw_loop:
                # timing mode: weights loaded once, then a device-side loop
                # of `hw_loop` iterations, each running `reps` rep bodies.
                load_weights()
                rep_body(False)
                with tc.For_i(0, hw_loop, 1):
                    for _ in range(reps):
                        rep_body(False)
            else:
                for rep in range(reps):
                    rep_body(rep == 0)

    nc.compile()
    return nc


def _get_nc(b0, b1, b2, b3, reps=1, chunks=(1024, 1024), hw_loop=0):
    key = (b0, b1, b2, b3, reps, tuple(chunks), hw_loop)
    if key not in _cache:
        _cache[key] = _build(b0, b1, b2, b3, reps, chunks, hw_loop)
    return _cache[key]


def _prep_inputs(z, bounds, W1, c1, W2, c2, W3, c3, W4, c4, W5, c5):
    import ml_dtypes
    bft = ml_dtypes.bfloat16

    b = np.asarray(bounds, np.float32)
    W1m = np.ascontiguousarray(W1[:Z], np.float32).astype(bft)
    b1 = (np.asarray(c1, np.float32)
          + b @ np.asarray(W1[Z:], np.float32)).astype(np.float32)

    wk = np.zeros((P, _WKW), bft)
    wk[:, _W2O:_W2O + 1024] = (np.asarray(W2, np.float32)
                               .reshape(4, P, 256).transpose(1, 0, 2)
                               .reshape(P, 1024).astype(bft))
    wk[:, _W3O:_W3O + 256] = (np.asarray(W3, np.float32)
                              .reshape(2, P, 128).transpose(1, 0, 2)
                              .reshape(P, 256).astype(bft))
    wk[:, _W4O:_W4O + 64] = np.asarray(W4, np.float32).astype(bft)
    w5a = np.concatenate(
        [np.asarray(W5, np.float32), np.asarray(c5, np.float32)[None, :]], 0)
    qidx = np.arange(256)
    gq, c2q, oq = qidx // 128, (qidx // 64) % 2, qidx % 64
    w5a = w5a[:, oq * 4 + 2 * gq + c2q]
    wk[0:65, _W5O:_W5O + 256] = w5a.astype(bft)

    bia = np.zeros((P, 8), np.float32)
    bia[:, 0:4] = b1.reshape(4, P).T
    bia[:, 4:6] = np.asarray(c2, np.float32).reshape(2, P).T
    bia[:, 6] = np.asarray(c3, np.float32)
    bia[0:64, 7] = np.asarray(c4, np.float32)

    zT = np.ascontiguousarray(np.asarray(z, np.float32).T).astype(bft)
    common = {"w1": W1m, "wk": wk, "bia": bia}
    in_maps = []
    for i in range(NCORES):
        m = dict(common)
        m["zt"] = np.ascontiguousarray(zT[:, i * BSC:(i + 1) * BSC])
        in_maps.append(m)
    return in_maps, (float(b[0]), float(b[1]), float(b[2]), float(b[3]))


def kernel(z, bounds, W1, c1, W2, c2, W3, c3, W4, c4, W5, c5):
    from concourse.bass_utils import run_bass_kernel_spmd

    in_maps, bvals = _prep_inputs(z, bounds, W1, c1, W2, c2, W3, c3,
                                  W4, c4, W5, c5)
    nc = _get_nc(*bvals)
    res = run_bass_kernel_spmd(nc, in_maps, core_ids=list(range(NCORES)))
    out = np.concatenate(
        [np.asarray(r["o"], np.float32) for r in res.results], axis=0)
    return (out.reshape(BS, 2, 2, NOBJ).transpose(0, 3, 1, 2)
            .reshape(BS, NOBJ, 4))


# revision 29
# speedup vs baseline: 1.6769x; 1.6472x over previous
"""Trainium2 Bass kernel for nn_CVX_Reasoning_Engine.

MLP (16384x512 -> 512 -> 256 -> 128 -> 64 -> 256) with LeakyReLU(0.2),
followed by a closed-form per-object/axis QP solve.

Strategy (v2, bf16):
- Pure data parallel over 8 NeuronCores (2048 batch rows each).
- All matmuls in bf16 (1 col/cycle on PE, FWL halves LDWEIGHTS); z is
  shipped bf16 (halves input DMA); output is stored fp16 (halves
  output DMA); QP runs in fp16 on DVE (2x mode).
- Host-side prep: fold `bounds` contribution of the concat into layer-1
  bias; transpose z so activations flow feature-major on-chip; append
  the layer-5 bias as an extra ones-row of the last activation (K=65
  matmul) so layer 5 exits batch-major.
- Elementwise work is spread across engines so none exceeds PE time:
  ACT does Prelu+bias for half the regions; the rest get a Pool
  in-place bias-add on PSUM followed by a single DVE op
  lrelu(v) = max(0.2*v, v).
- QP closed form without branches:
    g0 = max(pg, 1); s0 = max(pp, lo) + g0; w = min(s0, hi)
    u  = min(0.5*(pp - pg + hi), hi-1)
    x  = max(min(max(pp, lo), u), lo)
"""

import numpy as np

BS, Z, NOBJ = 16384, 512, 64
NCORES = 8
BSC = BS // NCORES            # 2048 batch rows per core
P = 128

# wk (bf16) packed offsets, in elements per partition
_W2O, _W3O, _W4O, _W5O = 0, 1024, 1280, 1344
_WKW = 1600

# bisection knobs
CFG = {
    "dve_acts": False,   # offload some lrelu+bias regions to DVE 2-op path
    "qp_f16": True,     # QP in fp16 (else fp32)
    "ablate": set(),     # subset of {"zload","acts","l5copy","qp","store"}
    "qp_ops": 7,         # how many of the 7 QP ops to emit (prefix)
    "qp_pool": False,    # route ops 1,3,5 to Pool (else all DVE; Pool ops
                         # measured ~10x slower than DVE on this HW)
}

_cache = {}


def _build(b0, b1, b2, b3, reps=1, chunks=(1024, 1024), hw_loop=0):
    import concourse.tile as tile
    from concourse import bacc, mybir

    f32 = mybir.dt.float32
    bf16 = mybir.dt.bfloat16
    f16 = mybir.dt.float16
    AF = mybir.ActivationFunctionType
    Alu = mybir.AluOpType

    nc = bacc.Bacc("TRN2", target_bir_lowering=False, debug=False,
                   num_devices=NCORES)

    zt_d = nc.dram_tensor("zt", (Z, BSC), bf16, kind="ExternalInput").ap()
    w1_d = nc.dram_tensor("w1", (512, 512), bf16, kind="ExternalInput").ap()
    wk_d = nc.dram_tensor("wk", (P, _WKW), bf16, kind="ExternalInput").ap()
    bia_d = nc.dram_tensor("bia", (P, 8), f32, kind="ExternalInput").ap()
    o_d = nc.dram_tensor("o", (BSC, 256), f16, kind="ExternalOutput").ap()

    lo_x, hi_x = float(b0), float(b2)
    lo_y, hi_y = float(b1), float(b3)

    with tile.TileContext(nc) as tc:
        with (
            tc.tile_pool(name="wp", bufs=1) as wp,
            tc.tile_pool(name="zp", bufs=2) as zp,
            tc.tile_pool(name="hp", bufs=2) as hp,
            tc.tile_pool(name="stg", bufs=3) as stg,
            tc.tile_pool(name="scp", bufs=2) as scp,
            tc.tile_pool(name="tmp", bufs=2) as tmp,
            tc.tile_pool(name="big", bufs=3, space="PSUM") as big,
            tc.tile_pool(name="ps5", bufs=2, space="PSUM") as ps5p,
        ):
            # ---- resident weights ----
            w1_sb = wp.tile([P, 4 * 512], bf16, tag="w1")
            w1v = w1_d.rearrange("(k p) m -> p k m", p=P)
            wk_sb = wp.tile([P, _WKW], bf16, tag="wk")
            bia_sb = wp.tile([P, 8], f32, tag="bia")

            def w1k(k):
                return w1_sb[:, k * 512:(k + 1) * 512]

            w2v = wk_sb[:, _W2O:_W2O + 1024]
            w3v = wk_sb[:, _W3O:_W3O + 256]
            w4v = wk_sb[:, _W4O:_W4O + 64]
            w5v = wk_sb[:, _W5O:_W5O + 256]

            def bia(c):
                return bia_sb[:, c:c + 1]

            ones_sb = wp.tile([1, max(chunks)], bf16, tag="ones")
            nc.vector.memset(ones_sb[:], 1.0)

            def load_weights():
                for k in range(4):
                    nc.sync.dma_start(w1_sb[:, k * 512:(k + 1) * 512],
                                      w1v[:, k, :])
                nc.sync.dma_start(wk_sb[:], wk_d)
                nc.sync.dma_start(bia_sb[:], bia_d)

            def rep_body(first_rep):
              col0 = 0
              for ci, W in enumerate(chunks):
                first = (first_rep and ci == 0)
                hfs = []
                off = 0
                while off < W:
                    hw = min(512, W - off)
                    hfs.append((off, hw))
                    off += hw

                # ---- load z chunk (feature-major, per-k split on chunk 0) ----
                zt_n = zp.tile([P, 4 * W], bf16, tag="zt")
                if "zload" in CFG["ablate"]:
                    nc.gpsimd.memset(zt_n[0:1, 0:2], 0.0)
                elif first:
                    for k in range(4):
                        nc.sync.dma_start(w1_sb[:, k * 512:(k + 1) * 512],
                                          w1v[:, k, :])
                        nc.sync.dma_start(
                            zt_n[:, k * W:(k + 1) * W],
                            zt_d[k * P:(k + 1) * P, col0:col0 + W])
                    nc.sync.dma_start(wk_sb[:], wk_d)
                    nc.sync.dma_start(bia_sb[:], bia_d)
                else:
                    nc.sync.dma_start(
                        zt_n[:].rearrange("p (k c) -> p k c", k=4),
                        zt_d[:, col0:col0 + W]
                            .rearrange("(k p) c -> p k c", p=P))

                # helper: write h = lrelu(psum + bias) into dst.
                # mode "act": one ACT op (Prelu with bias).
                # mode "dve": DVE adds bias PSUM -> SBUF f32 scratch, then
                #             one DVE op lrelu(v) = max(0.2*v, v). (GPSIMD
                #             cannot read PSUM nor run TensorScalarPtr/STT.)
                def act_or_dve(dst, pst_v, b_ap, mode, W=W):
                    if "acts" in CFG["ablate"]:
                        return
                    if not CFG["dve_acts"]:
                        mode = "act"
                    if mode == "act":
                        nc.scalar.activation(dst, pst_v, AF.Prelu,
                                             bias=b_ap, alpha=0.2)
                    else:
                        sc = scp.tile([P, W], f32, tag="sc")
                        nc.vector.tensor_scalar(sc[:], pst_v, b_ap, None,
                                                Alu.add)
                        nc.vector.scalar_tensor_tensor(
                            dst, sc[:], 0.2, sc[:], Alu.mult, Alu.max)

                abl = CFG["ablate"]
                # ---- L1: 512 -> 512 ----
                h1_n = zt_n if "acts" in abl else hp.tile(
                    [P, 4 * W], bf16, tag="h1")
                for m in range(4):
                    pst = big.tile([P, W], f32, tag="big")
                    for k in range(4):
                        for off, hw in hfs:
                            nc.tensor.matmul(
                                pst[:, off:off + hw],
                                w1k(k)[:, m * 128:(m + 1) * 128],
                                zt_n[:, k * W + off:k * W + off + hw],
                                start=(k == 0), stop=(k == 3))
                    act_or_dve(h1_n[:, m * W:(m + 1) * W], pst[:, 0:W],
                               bia(m), "act" if m < 3 else "dve")

                # ---- L2: 512 -> 256 ----
                h2_n = zt_n if "acts" in abl else hp.tile(
                    [P, 2 * W], bf16, tag="h2")
                for m in range(2):
                    pst = big.tile([P, W], f32, tag="big")
                    for k in range(4):
                        for off, hw in hfs:
                            nc.tensor.matmul(
                                pst[:, off:off + hw],
                                w2v[:, k * 256 + m * 128:k * 256 + (m + 1) * 128],
                                h1_n[:, k * W + off:k * W + off + hw],
                                start=(k == 0), stop=(k == 3))
                    act_or_dve(h2_n[:, m * W:(m + 1) * W], pst[:, 0:W],
                               bia(4 + m), "act" if m == 0 else "dve")

                # ---- L3: 256 -> 128 ----
                h3_n = zt_n if "acts" in abl else hp.tile(
                    [P, W], bf16, tag="h3")
                pst = big.tile([P, W], f32, tag="big")
                for k in range(2):
                    for off, hw in hfs:
                        nc.tensor.matmul(
                            pst[:, off:off + hw],
                            w3v[:, k * 128:(k + 1) * 128],
                            h2_n[:, k * W + off:k * W + off + hw],
                            start=(k == 0), stop=(k == 1))
                if "acts" not in CFG["ablate"]:
                    nc.scalar.activation(h3_n[:], pst[:, 0:W], AF.Prelu,
                                         bias=bia(6), alpha=0.2)

                # ---- L4: 128 -> 64 (plus ones row for L5 bias) ----
                h4_n = zt_n if "acts" in abl else hp.tile(
                    [65, W], bf16, tag="h4")
                pst = big.tile([P, W], f32, tag="big")
                for off, hw in hfs:
                    nc.tensor.matmul(pst[0:64, off:off + hw],
                                     w4v[:], h3_n[:, off:off + hw],
                                     start=True, stop=True)
                if "acts" not in CFG["ablate"]:
                    nc.scalar.activation(h4_n[0:64, :], pst[0:64, 0:W],
                                         AF.Prelu, bias=bia(7)[0:64],
                                         alpha=0.2)
                    nc.sync.dma_start(h4_n[64:65, :], ones_sb[0:1, 0:W])

                # ---- L5 + QP per double-staging (512 batch rows) ----
                nds = W // 512
                for ds in range(nds):
                    fq = f16 if CFG["qp_f16"] else f32
                    need_p = "l5copy" not in abl or "qp" not in abl
                    need_o = "qp" not in abl or "store" not in abl
                    p_sb = o_sb = None
                    if need_p:
                        p_sb = stg.tile([P, 1024], fq, tag="p")
                    if need_o:
                        o_sb = stg.tile([P, 1024], f16, tag="o")
                    # W5's columns are host-permuted to [g][c2][o] order
                    # (g=0: position params x,y; g=1: size params w,h), so
                    # p_sb laid out [g=2][s=4][c2=2][o=64] gives the QP
                    # fully contiguous pp/pg operand blocks.
                    for st2 in range(2):
                        p5 = ps5p.tile([P, 512], f32, tag="l5")
                        for j in range(2):
                            sub = ds * 4 + st2 * 2 + j
                            nc.tensor.matmul(
                                p5[:, j * 256:(j + 1) * 256],
                                h4_n[0:65, sub * P:(sub + 1) * P],
                                w5v[0:65, :], start=True, stop=True)
                        if "l5copy" not in CFG["ablate"]:
                            dst = (p_sb[:]
                                   .rearrange("p (g s f) -> p g s f",
                                              g=2, s=4)
                                   [:, :, st2 * 2:(st2 + 1) * 2, :])
                            src = p5[:].rearrange(
                                "p (s2 g f) -> p g s2 f", s2=2, g=2)
                            nc.vector.tensor_copy(dst, src)

                    # QP solve (batch-major, fp16, contiguous blocks)
                    S = 4
                    if b0 == b1 and b2 == b3:
                        groups = [((0, 1), 2, lo_x, hi_x)]
                    else:
                        groups = [((0, 0), 1, lo_x, hi_x),
                                  ((1, 1), 1, lo_y, hi_y)]
                    if "qp" in CFG["ablate"]:
                        groups = []
                    for (c2a, c2b), cw, lo, hi in groups:
                        fd = S * NOBJ * cw
                        if cw == 2:
                            pp = p_sb[:, 0:512]
                            pg = p_sb[:, 512:1024]
                            ovv = o_sb[:].rearrange("p (s q) -> p s q", s=S)
                            ox = ovv[:, :, 0:128]
                            ow = ovv[:, :, 128:256]
                        else:
                            pv5 = p_sb[:].rearrange(
                                "p (g s c2 o) -> p g s (c2 o)", g=2, s=4)
                            pp = pv5[:, 0, :, c2a * 64:(c2a + 1) * 64]
                            pg = pv5[:, 1, :, c2a * 64:(c2a + 1) * 64]
                            ovv = o_sb[:].rearrange(
                                "p (s g c2 o) -> p s g (c2 o)", s=4, g=2)
                            ox = ovv[:, :, 0, c2a * 64:(c2a + 1) * 64]
                            ow = ovv[:, :, 1, c2a * 64:(c2a + 1) * 64]

                        g0 = tmp.tile([P, fd], fq, tag="g0")
                        s0 = tmp.tile([P, fd], fq, tag="s0")
                        u = tmp.tile([P, fd], fq, tag="u")
                        t1 = tmp.tile([P, fd], fq, tag="t1")

                        def tv(t, fd=fd):
                            return t[:, 0:fd].rearrange(
                                "p (s q) -> p s q", s=S)

                        mk = (lambda t: t[:]) if cw == 2 else tv
                        eng2 = nc.gpsimd if CFG["qp_pool"] else nc.vector
                        nops = CFG["qp_ops"]

                        # g0 = max(pg, 1); s0 = max(pp, lo) + g0; w = min(s0, hi)
                        if nops >= 1:
                            eng2.tensor_scalar_max(mk(g0), pg, 1.0)
                        if nops >= 2:
                            nc.vector.scalar_tensor_tensor(
                                mk(s0), pp, lo, mk(g0), Alu.max, Alu.add)
                        if nops >= 3:
                            eng2.tensor_scalar_min(ow, tv(s0), hi)
                        # u = (pp + hi) - pg ; scale+clip
                        if nops >= 4:
                            nc.vector.scalar_tensor_tensor(
                                mk(u), pp, hi, pg, Alu.add, Alu.subtract)
                        if nops >= 5:
                            eng2.tensor_scalar(mk(u), mk(u), 0.5, hi - 1.0,
                                               Alu.mult, Alu.min)
                        # x = max(min(max(pp, lo), u), lo)
                        if nops >= 6:
                            nc.vector.scalar_tensor_tensor(
                                mk(t1), pp, lo, mk(u), Alu.max, Alu.min)
                        if nops >= 7:
                            nc.vector.tensor_scalar_max(ox, tv(t1), lo)

                    # ---- store double-staging -> DRAM (contiguous rows) ----
                    r0 = col0 + ds * 512
                    if "store" not in CFG["ablate"]:
                        o_src = o_sb if "qp" not in abl else p_sb
                        nc.sync.dma_start(
                            o_d[r0:r0 + 512, :]
                                .rearrange("(s p) f -> p s f", p=P),
                            o_src[:].rearrange("p (s f) -> p s f", s=4))
                col0 += W

            if hw_loop:
                # timing mode: weights loaded once, then a device-side loop
                # of `hw_loop` iterations, each running `reps` rep bodies.
                load_weights()
                rep_body(False)
                with tc.For_i(0, hw_loop, 1):
                    for _ in range(reps):
                        rep_body(False)
            else:
                for rep in range(reps):
                    rep_body(rep == 0)

    nc.compile()
    return nc


def _get_nc(b0, b1, b2, b3, reps=1, chunks=(1024, 1024), hw_loop=0):
    key = (b0, b1, b2, b3, reps, tuple(chunks), hw_loop)
    if key not in _cache:
        _cache[key] = _build(b0, b1, b2, b3, reps, chunks, hw_loop)
    return _cache[key]


def _prep_inputs(z, bounds, W1, c1, W2, c2, W3, c3, W4, c4, W5, c5):
    import ml_dtypes
    bft = ml_dtypes.bfloat16

    b = np.asarray(bounds, np.float32)
    W1m = np.ascontiguousarray(W1[:Z], np.float32).astype(bft)
    b1 = (np.asarray(c1, np.float32)
          + b @ np.asarray(W1[Z:], np.float32)).astype(np.float32)

    wk = np.zeros((P, _WKW), bft)
    wk[:, _W2O:_W2O + 1024] = (np.asarray(W2, np.float32)
                               .reshape(4, P, 256).transpose(1, 0, 2)
                               .reshape(P, 1024).astype(bft))
    wk[:, _W3O:_W3O + 256] = (np.asarray(W3, np.float32)
                              .reshape(2, P, 128).transpose(1, 0, 2)
                              .reshape(P, 256).astype(bft))
    wk[:, _W4O:_W4O + 64] = np.asarray(W4, np.float32).astype(bft)
    w5a = np.concatenate(
        [np.asarray(W5, np.float32), np.asarray(c5, np.float32)[None, :]], 0)
    qidx = np.arange(256)
    gq, c2q, oq = qidx // 128, (qidx // 64) % 2, qidx % 64
    w5a = w5a[:, oq * 4 + 2 * gq + c2q]
    wk[0:65, _W5O:_W5O + 256] = w5a.astype(bft)

    bia = np.zeros((P, 8), np.float32)
    bia[:, 0:4] = b1.reshape(4, P).T
    bia[:, 4:6] = np.asarray(c2, np.float32).reshape(2, P).T
    bia[:, 6] = np.asarray(c3, np.float32)
    bia[0:64, 7] = np.asarray(c4, np.float32)

    zT = np.ascontiguousarray(np.asarray(z, np.float32).T).astype(bft)
    common = {"w1": W1m, "wk": wk, "bia": bia}
    in_maps = []
    for i in range(NCORES):
        m = dict(common)
        m["zt"] = np.ascontiguousarray(zT[:, i * BSC:(i + 1) * BSC])
        in_maps.append(m)
    return in_maps, (float(b[0]), float(b[1]), float(b[2]), float(b[3]))


def kernel(z, bounds, W1, c1, W2, c2, W3, c3, W4, c4, W5, c5):
    from concourse.bass_utils import run_bass_kernel_spmd

    in_maps, bvals = _prep_inputs(z, bounds, W1, c1, W2, c2, W3, c3,
                                  W4, c4, W5, c5)
    nc = _get_nc(*bvals)
    res = run_bass_kernel_spmd(nc, in_maps, core_ids=list(range(NCORES)))
    out = np.concatenate(
        [np.asarray(r["o"], np.float32) for r in res.results], axis=0)
    return (out.reshape(BS, 2, 2, NOBJ).transpose(0, 3, 1, 2)
            .reshape(BS, NOBJ, 4))


# revision 31
# speedup vs baseline: 3.0803x; 1.8368x over previous
"""Trainium2 Bass kernel for nn_CVX_Reasoning_Engine.

MLP (16384x512 -> 512 -> 256 -> 128 -> 64 -> 256) with LeakyReLU(0.2),
followed by a closed-form per-object/axis QP solve.

Strategy (v2, bf16):
- Pure data parallel over 8 NeuronCores (2048 batch rows each).
- All matmuls in bf16 (1 col/cycle on PE, FWL halves LDWEIGHTS); z is
  shipped bf16 (halves input DMA); output is stored fp16 (halves
  output DMA); QP runs in fp16 on DVE (2x mode).
- Host-side prep: fold `bounds` contribution of the concat into layer-1
  bias; transpose z so activations flow feature-major on-chip; append
  the layer-5 bias as an extra ones-row of the last activation (K=65
  matmul) so layer 5 exits batch-major.
- Elementwise work is spread across engines so none exceeds PE time:
  ACT does Prelu+bias for half the regions; the rest get a Pool
  in-place bias-add on PSUM followed by a single DVE op
  lrelu(v) = max(0.2*v, v).
- QP closed form without branches:
    g0 = max(pg, 1); s0 = max(pp, lo) + g0; w = min(s0, hi)
    u  = min(0.5*(pp - pg + hi), hi-1)
    x  = max(min(max(pp, lo), u), lo)
"""

import numpy as np

BS, Z, NOBJ = 16384, 512, 64
NCORES = 8
BSC = BS // NCORES            # 2048 batch rows per core
P = 128

# wk (bf16) packed offsets, in elements per partition
_W2O, _W3O, _W4O, _W5O = 0, 1024, 1280, 1344
_WKW = 1600

# bisection knobs
CFG = {
    "dve_acts": False,   # offload some lrelu+bias regions to DVE 2-op path
    "qp_f16": True,     # QP in fp16 (else fp32)
    "ablate": set(),     # subset of {"zload","acts","l5copy","qp","store"}
    "qp_ops": 7,         # how many of the 7 QP ops to emit (prefix)
    "qp_pool": False,    # route ops 1,3,5 to Pool (else all DVE; Pool ops
                         # measured ~10x slower than DVE on this HW)
}

_cache = {}


def _build(b0, b1, b2, b3, reps=1, chunks=(2048,), hw_loop=0):
    import concourse.tile as tile
    from concourse import bacc, mybir

    f32 = mybir.dt.float32
    bf16 = mybir.dt.bfloat16
    f16 = mybir.dt.float16
    AF = mybir.ActivationFunctionType
    Alu = mybir.AluOpType

    nc = bacc.Bacc("TRN2", target_bir_lowering=False, debug=False,
                   num_devices=NCORES)

    zt_d = nc.dram_tensor("zt", (Z, BSC), bf16, kind="ExternalInput").ap()
    w1_d = nc.dram_tensor("w1", (512, 512), bf16, kind="ExternalInput").ap()
    wk_d = nc.dram_tensor("wk", (P, _WKW), bf16, kind="ExternalInput").ap()
    bia_d = nc.dram_tensor("bia", (P, 8), f32, kind="ExternalInput").ap()
    o_d = nc.dram_tensor("o", (BSC, 256), f16, kind="ExternalOutput").ap()

    lo_x, hi_x = float(b0), float(b2)
    lo_y, hi_y = float(b1), float(b3)

    with tile.TileContext(nc) as tc:
        with (
            tc.tile_pool(name="wp", bufs=1) as wp,
            tc.tile_pool(name="zp", bufs=2) as zp,
            tc.tile_pool(name="hp", bufs=2) as hp,
            tc.tile_pool(name="stg", bufs=3) as stg,
            tc.tile_pool(name="scp", bufs=2) as scp,
            tc.tile_pool(name="tmp", bufs=2) as tmp,
            tc.tile_pool(name="big", bufs=3, space="PSUM") as big,
            tc.tile_pool(name="ps5", bufs=2, space="PSUM") as ps5p,
        ):
            # ---- resident weights ----
            w1_sb = wp.tile([P, 4 * 512], bf16, tag="w1")
            w1v = w1_d.rearrange("(k p) m -> p k m", p=P)
            wk_sb = wp.tile([P, _WKW], bf16, tag="wk")
            bia_sb = wp.tile([P, 8], f32, tag="bia")

            def w1k(k):
                return w1_sb[:, k * 512:(k + 1) * 512]

            w2v = wk_sb[:, _W2O:_W2O + 1024]
            w3v = wk_sb[:, _W3O:_W3O + 256]
            w4v = wk_sb[:, _W4O:_W4O + 64]
            w5v = wk_sb[:, _W5O:_W5O + 256]

            def bia(c):
                return bia_sb[:, c:c + 1]

            ones_sb = wp.tile([1, max(chunks)], bf16, tag="ones")
            nc.vector.memset(ones_sb[:], 1.0)

            def load_weights():
                for k in range(4):
                    nc.sync.dma_start(w1_sb[:, k * 512:(k + 1) * 512],
                                      w1v[:, k, :])
                nc.sync.dma_start(wk_sb[:], wk_d)
                nc.sync.dma_start(bia_sb[:], bia_d)

            def rep_body(first_rep):
              col0 = 0
              for ci, W in enumerate(chunks):
                first = (first_rep and ci == 0)
                hfs = []
                off = 0
                while off < W:
                    hw = min(512, W - off)
                    hfs.append((off, hw))
                    off += hw

                # ---- load z chunk (feature-major, per-k split on chunk 0) ----
                zt_n = zp.tile([P, 4 * W], bf16, tag="zt")
                if "zload" in CFG["ablate"]:
                    nc.gpsimd.memset(zt_n[0:1, 0:2], 0.0)
                elif first:
                    for k in range(4):
                        nc.sync.dma_start(w1_sb[:, k * 512:(k + 1) * 512],
                                          w1v[:, k, :])
                        nc.sync.dma_start(
                            zt_n[:, k * W:(k + 1) * W],
                            zt_d[k * P:(k + 1) * P, col0:col0 + W])
                    nc.sync.dma_start(wk_sb[:], wk_d)
                    nc.sync.dma_start(bia_sb[:], bia_d)
                else:
                    nc.sync.dma_start(
                        zt_n[:].rearrange("p (k c) -> p k c", k=4),
                        zt_d[:, col0:col0 + W]
                            .rearrange("(k p) c -> p k c", p=P))

                # helper: write h = lrelu(psum + bias) into dst.
                # mode "act": one ACT op (Prelu with bias).
                # mode "dve": DVE adds bias PSUM -> SBUF f32 scratch, then
                #             one DVE op lrelu(v) = max(0.2*v, v). (GPSIMD
                #             cannot read PSUM nor run TensorScalarPtr/STT.)
                def act_or_dve(dst, pst_v, b_ap, mode, W=W):
                    if "acts" in CFG["ablate"]:
                        return
                    if not CFG["dve_acts"]:
                        mode = "act"
                    if mode == "act":
                        nc.scalar.activation(dst, pst_v, AF.Prelu,
                                             bias=b_ap, alpha=0.2)
                    else:
                        sc = scp.tile([P, W], f32, tag="sc")
                        nc.vector.tensor_scalar(sc[:], pst_v, b_ap, None,
                                                Alu.add)
                        nc.vector.scalar_tensor_tensor(
                            dst, sc[:], 0.2, sc[:], Alu.mult, Alu.max)

                abl = CFG["ablate"]
                HB = min(W, 1024)       # psum tile width (<= 2 banks)
                NH = W // HB            # halves per m-tile
                # ---- L1: 512 -> 512 ----
                h1_n = zt_n if "acts" in abl else hp.tile(
                    [P, 4 * W], bf16, tag="h1")
                for m in range(4):
                    psts = [big.tile([P, HB], f32, tag="big",
                                     name=f"pst{h}") for h in range(NH)]
                    for k in range(4):
                        for h in range(NH):
                            for o2 in range(0, HB, 512):
                                nc.tensor.matmul(
                                    psts[h][:, o2:o2 + 512],
                                    w1k(k)[:, m * 128:(m + 1) * 128],
                                    zt_n[:, k * W + h * HB + o2:
                                         k * W + h * HB + o2 + 512],
                                    start=(k == 0), stop=(k == 3))
                    for h in range(NH):
                        act_or_dve(
                            h1_n[:, m * W + h * HB:m * W + (h + 1) * HB],
                            psts[h][:], bia(m), "act")

                # ---- L2: 512 -> 256 ----
                h2_n = zt_n if "acts" in abl else hp.tile(
                    [P, 2 * W], bf16, tag="h2")
                for m in range(2):
                    psts = [big.tile([P, HB], f32, tag="big",
                                     name=f"pst{h}") for h in range(NH)]
                    for k in range(4):
                        for h in range(NH):
                            for o2 in range(0, HB, 512):
                                nc.tensor.matmul(
                                    psts[h][:, o2:o2 + 512],
                                    w2v[:, k * 256 + m * 128:
                                        k * 256 + (m + 1) * 128],
                                    h1_n[:, k * W + h * HB + o2:
                                         k * W + h * HB + o2 + 512],
                                    start=(k == 0), stop=(k == 3))
                    for h in range(NH):
                        act_or_dve(
                            h2_n[:, m * W + h * HB:m * W + (h + 1) * HB],
                            psts[h][:], bia(4 + m), "act")

                # ---- L3: 256 -> 128 ----
                h3_n = zt_n if "acts" in abl else hp.tile(
                    [P, W], bf16, tag="h3")
                psts = [big.tile([P, HB], f32, tag="big",
                                 name=f"pst{h}") for h in range(NH)]
                for k in range(2):
                    for h in range(NH):
                        for o2 in range(0, HB, 512):
                            nc.tensor.matmul(
                                psts[h][:, o2:o2 + 512],
                                w3v[:, k * 128:(k + 1) * 128],
                                h2_n[:, k * W + h * HB + o2:
                                     k * W + h * HB + o2 + 512],
                                start=(k == 0), stop=(k == 1))
                if "acts" not in CFG["ablate"]:
                    for h in range(NH):
                        nc.scalar.activation(
                            h3_n[:, h * HB:(h + 1) * HB], psts[h][:],
                            AF.Prelu, bias=bia(6), alpha=0.2)

                # ---- L4: 128 -> 64 (plus ones row for L5 bias) ----
                h4_n = zt_n if "acts" in abl else hp.tile(
                    [65, W], bf16, tag="h4")
                psts = [big.tile([P, HB], f32, tag="big",
                                 name=f"pst{h}") for h in range(NH)]
                for h in range(NH):
                    for o2 in range(0, HB, 512):
                        nc.tensor.matmul(
                            psts[h][0:64, o2:o2 + 512], w4v[:],
                            h3_n[:, h * HB + o2:h * HB + o2 + 512],
                            start=True, stop=True)
                if "acts" not in CFG["ablate"]:
                    for h in range(NH):
                        nc.scalar.activation(
                            h4_n[0:64, h * HB:(h + 1) * HB],
                            psts[h][0:64, :], AF.Prelu,
                            bias=bia(7)[0:64], alpha=0.2)
                    nc.sync.dma_start(h4_n[64:65, :], ones_sb[0:1, 0:W])

                # ---- L5 + QP per double-staging (512 batch rows) ----
                nds = W // 512
                for ds in range(nds):
                    fq = f16 if CFG["qp_f16"] else f32
                    need_p = "l5copy" not in abl or "qp" not in abl
                    need_o = "qp" not in abl or "store" not in abl
                    p_sb = o_sb = None
                    if need_p:
                        p_sb = stg.tile([P, 1024], fq, tag="p")
                    if need_o:
                        o_sb = stg.tile([P, 1024], f16, tag="o")
                    # W5's columns are host-permuted to [g][c2][o] order
                    # (g=0: position params x,y; g=1: size params w,h), so
                    # p_sb laid out [g=2][s=4][c2=2][o=64] gives the QP
                    # fully contiguous pp/pg operand blocks.
                    for st2 in range(2):
                        p5 = ps5p.tile([P, 512], f32, tag="l5")
                        for j in range(2):
                            sub = ds * 4 + st2 * 2 + j
                            nc.tensor.matmul(
                                p5[:, j * 256:(j + 1) * 256],
                                h4_n[0:65, sub * P:(sub + 1) * P],
                                w5v[0:65, :], start=True, stop=True)
                        if "l5copy" not in CFG["ablate"]:
                            dst = (p_sb[:]
                                   .rearrange("p (g s f) -> p g s f",
                                              g=2, s=4)
                                   [:, :, st2 * 2:(st2 + 1) * 2, :])
                            src = p5[:].rearrange(
                                "p (s2 g f) -> p g s2 f", s2=2, g=2)
                            nc.vector.tensor_copy(dst, src)

                    # QP solve (batch-major, fp16, contiguous blocks)
                    S = 4
                    if b0 == b1 and b2 == b3:
                        groups = [((0, 1), 2, lo_x, hi_x)]
                    else:
                        groups = [((0, 0), 1, lo_x, hi_x),
                                  ((1, 1), 1, lo_y, hi_y)]
                    if "qp" in CFG["ablate"]:
                        groups = []
                    for (c2a, c2b), cw, lo, hi in groups:
                        fd = S * NOBJ * cw
                        if cw == 2:
                            pp = p_sb[:, 0:512]
                            pg = p_sb[:, 512:1024]
                            ovv = o_sb[:].rearrange("p (s q) -> p s q", s=S)
                            ox = ovv[:, :, 0:128]
                            ow = ovv[:, :, 128:256]
                        else:
                            pv5 = p_sb[:].rearrange(
                                "p (g s c2 o) -> p g s (c2 o)", g=2, s=4)
                            pp = pv5[:, 0, :, c2a * 64:(c2a + 1) * 64]
                            pg = pv5[:, 1, :, c2a * 64:(c2a + 1) * 64]
                            ovv = o_sb[:].rearrange(
                                "p (s g c2 o) -> p s g (c2 o)", s=4, g=2)
                            ox = ovv[:, :, 0, c2a * 64:(c2a + 1) * 64]
                            ow = ovv[:, :, 1, c2a * 64:(c2a + 1) * 64]

                        g0 = tmp.tile([P, fd], fq, tag="g0")
                        s0 = tmp.tile([P, fd], fq, tag="s0")
                        u = tmp.tile([P, fd], fq, tag="u")
                        t1 = tmp.tile([P, fd], fq, tag="t1")

                        def tv(t, fd=fd):
                            return t[:, 0:fd].rearrange(
                                "p (s q) -> p s q", s=S)

                        mk = (lambda t: t[:]) if cw == 2 else tv
                        eng2 = nc.gpsimd if CFG["qp_pool"] else nc.vector
                        nops = CFG["qp_ops"]

                        # g0 = max(pg, 1); s0 = max(pp, lo) + g0; w = min(s0, hi)
                        if nops >= 1:
                            eng2.tensor_scalar_max(mk(g0), pg, 1.0)
                        if nops >= 2:
                            nc.vector.scalar_tensor_tensor(
                                mk(s0), pp, lo, mk(g0), Alu.max, Alu.add)
                        if nops >= 3:
                            eng2.tensor_scalar_min(ow, tv(s0), hi)
                        # u = (pp + hi) - pg ; scale+clip
                        if nops >= 4:
                            nc.vector.scalar_tensor_tensor(
                                mk(u), pp, hi, pg, Alu.add, Alu.subtract)
                        if nops >= 5:
                            eng2.tensor_scalar(mk(u), mk(u), 0.5, hi - 1.0,
                                               Alu.mult, Alu.min)
                        # x = max(min(max(pp, lo), u), lo)
                        if nops >= 6:
                            nc.vector.scalar_tensor_tensor(
                                mk(t1), pp, lo, mk(u), Alu.max, Alu.min)
                        if nops >= 7:
                            nc.vector.tensor_scalar_max(ox, tv(t1), lo)

                    # ---- store double-staging -> DRAM (contiguous rows) ----
                    r0 = col0 + ds * 512
                    if "store" not in CFG["ablate"]:
                        o_src = o_sb if "qp" not in abl else p_sb
                        nc.sync.dma_start(
                            o_d[r0:r0 + 512, :]
                                .rearrange("(s p) f -> p s f", p=P),
                            o_src[:].rearrange("p (s f) -> p s f", s=4))
                col0 += W

            if hw_loop:
                # timing mode: weights loaded once, then a device-side loop
                # of `hw_loop` iterations, each running `reps` rep bodies.
                load_weights()
                rep_body(False)
                with tc.For_i(0, hw_loop, 1):
                    for _ in range(reps):
                        rep_body(False)
            else:
                for rep in range(reps):
                    rep_body(rep == 0)

    nc.compile()
    return nc


def _get_nc(b0, b1, b2, b3, reps=1, chunks=(2048,), hw_loop=0):
    key = (b0, b1, b2, b3, reps, tuple(chunks), hw_loop)
    if key not in _cache:
        _cache[key] = _build(b0, b1, b2, b3, reps, chunks, hw_loop)
    return _cache[key]


def _prep_inputs(z, bounds, W1, c1, W2, c2, W3, c3, W4, c4, W5, c5):
    import ml_dtypes
    bft = ml_dtypes.bfloat16

    b = np.asarray(bounds, np.float32)
    W1m = np.ascontiguousarray(W1[:Z], np.float32).astype(bft)
    b1 = (np.asarray(c1, np.float32)
          + b @ np.asarray(W1[Z:], np.float32)).astype(np.float32)

    wk = np.zeros((P, _WKW), bft)
    wk[:, _W2O:_W2O + 1024] = (np.asarray(W2, np.float32)
                               .reshape(4, P, 256).transpose(1, 0, 2)
                               .reshape(P, 1024).astype(bft))
    wk[:, _W3O:_W3O + 256] = (np.asarray(W3, np.float32)
                              .reshape(2, P, 128).transpose(1, 0, 2)
                              .reshape(P, 256).astype(bft))
    wk[:, _W4O:_W4O + 64] = np.asarray(W4, np.float32).astype(bft)
    w5a = np.concatenate(
        [np.asarray(W5, np.float32), np.asarray(c5, np.float32)[None, :]], 0)
    qidx = np.arange(256)
    gq, c2q, oq = qidx // 128, (qidx // 64) % 2, qidx % 64
    w5a = w5a[:, oq * 4 + 2 * gq + c2q]
    wk[0:65, _W5O:_W5O + 256] = w5a.astype(bft)

    bia = np.zeros((P, 8), np.float32)
    bia[:, 0:4] = b1.reshape(4, P).T
    bia[:, 4:6] = np.asarray(c2, np.float32).reshape(2, P).T
    bia[:, 6] = np.asarray(c3, np.float32)
    bia[0:64, 7] = np.asarray(c4, np.float32)

    zT = np.ascontiguousarray(np.asarray(z, np.float32).T).astype(bft)
    common = {"w1": W1m, "wk": wk, "bia": bia}
    in_maps = []
    for i in range(NCORES):
        m = dict(common)
        m["zt"] = np.ascontiguousarray(zT[:, i * BSC:(i + 1) * BSC])
        in_maps.append(m)
    return in_maps, (float(b[0]), float(b[1]), float(b[2]), float(b[3]))


def kernel(z, bounds, W1, c1, W2, c2, W3, c3, W4, c4, W5, c5):
    from concourse.bass_utils import run_bass_kernel_spmd

    in_maps, bvals = _prep_inputs(z, bounds, W1, c1, W2, c2, W3, c3,
                                  W4, c4, W5, c5)
    nc = _get_nc(*bvals)
    res = run_bass_kernel_spmd(nc, in_maps, core_ids=list(range(NCORES)))
    out = np.concatenate(
        [np.asarray(r["o"], np.float32) for r in res.results], axis=0)
    return (out.reshape(BS, 2, 2, NOBJ).transpose(0, 3, 1, 2)
            .reshape(BS, NOBJ, 4))


# revision 32
# speedup vs baseline: 3.1740x; 1.0304x over previous
"""Trainium2 Bass kernel for nn_CVX_Reasoning_Engine.

MLP (16384x512 -> 512 -> 256 -> 128 -> 64 -> 256) with LeakyReLU(0.2),
followed by a closed-form per-object/axis QP solve.

Strategy (v2, bf16):
- Pure data parallel over 8 NeuronCores (2048 batch rows each).
- All matmuls in bf16 (1 col/cycle on PE, FWL halves LDWEIGHTS); z is
  shipped bf16 (halves input DMA); output is stored fp16 (halves
  output DMA); QP runs in fp16 on DVE (2x mode).
- Host-side prep: fold `bounds` contribution of the concat into layer-1
  bias; transpose z so activations flow feature-major on-chip; append
  the layer-5 bias as an extra ones-row of the last activation (K=65
  matmul) so layer 5 exits batch-major.
- Elementwise work is spread across engines so none exceeds PE time:
  ACT does Prelu+bias for half the regions; the rest get a Pool
  in-place bias-add on PSUM followed by a single DVE op
  lrelu(v) = max(0.2*v, v).
- QP closed form without branches:
    g0 = max(pg, 1); s0 = max(pp, lo) + g0; w = min(s0, hi)
    u  = min(0.5*(pp - pg + hi), hi-1)
    x  = max(min(max(pp, lo), u), lo)
"""

import numpy as np

BS, Z, NOBJ = 16384, 512, 64
NCORES = 8
BSC = BS // NCORES            # 2048 batch rows per core
P = 128

# wk (bf16) packed offsets, in elements per partition
_W2O, _W3O, _W4O, _W5O = 0, 1024, 1280, 1344
_WKW = 1600

# bisection knobs
CFG = {
    "dve_acts": False,   # offload some lrelu+bias regions to DVE 2-op path
    "qp_f16": True,     # QP in fp16 (else fp32)
    "ablate": set(),     # subset of {"zload","acts","l5copy","qp","store"}
    "qp_ops": 7,         # how many of the 7 QP ops to emit (prefix)
    "qp_pool": False,    # route ops 1,3,5 to Pool (else all DVE; Pool ops
                         # measured ~10x slower than DVE on this HW)
}

_cache = {}


def _build(b0, b1, b2, b3, reps=1, chunks=(2048,), hw_loop=0):
    import concourse.tile as tile
    from concourse import bacc, mybir

    f32 = mybir.dt.float32
    bf16 = mybir.dt.bfloat16
    f16 = mybir.dt.float16
    AF = mybir.ActivationFunctionType
    Alu = mybir.AluOpType

    nc = bacc.Bacc("TRN2", target_bir_lowering=False, debug=False,
                   num_devices=NCORES)

    zt_d = nc.dram_tensor("zt", (Z, BSC), bf16, kind="ExternalInput").ap()
    w1_d = nc.dram_tensor("w1", (512, 512), bf16, kind="ExternalInput").ap()
    wk_d = nc.dram_tensor("wk", (P, _WKW), bf16, kind="ExternalInput").ap()
    bia_d = nc.dram_tensor("bia", (P, 8), f32, kind="ExternalInput").ap()
    o_d = nc.dram_tensor("o", (256, BSC), f16, kind="ExternalOutput").ap()

    lo_x, hi_x = float(b0), float(b2)
    lo_y, hi_y = float(b1), float(b3)

    with tile.TileContext(nc) as tc:
        with (
            tc.tile_pool(name="wp", bufs=1) as wp,
            tc.tile_pool(name="zp", bufs=2) as zp,
            tc.tile_pool(name="hp", bufs=2) as hp,
            tc.tile_pool(name="stg", bufs=3) as stg,
            tc.tile_pool(name="scp", bufs=2) as scp,
            tc.tile_pool(name="tmp", bufs=2) as tmp,
            tc.tile_pool(name="big", bufs=3, space="PSUM") as big,
            tc.tile_pool(name="ps5", bufs=2, space="PSUM") as ps5p,
        ):
            # ---- resident weights ----
            w1_sb = wp.tile([P, 4 * 512], bf16, tag="w1")
            w1v = w1_d.rearrange("(k p) m -> p k m", p=P)
            wk_sb = wp.tile([P, _WKW], bf16, tag="wk")
            bia_sb = wp.tile([P, 8], f32, tag="bia")

            def w1k(k):
                return w1_sb[:, k * 512:(k + 1) * 512]

            w2v = wk_sb[:, _W2O:_W2O + 1024]
            w3v = wk_sb[:, _W3O:_W3O + 256]
            w4v = wk_sb[:, _W4O:_W4O + 64]
            w5v = wk_sb[:, _W5O:_W5O + 256]

            def bia(c):
                return bia_sb[:, c:c + 1]

            ones_sb = wp.tile([1, max(chunks)], bf16, tag="ones")
            nc.vector.memset(ones_sb[:], 1.0)

            def load_weights():
                for k in range(4):
                    nc.sync.dma_start(w1_sb[:, k * 512:(k + 1) * 512],
                                      w1v[:, k, :])
                nc.sync.dma_start(wk_sb[:], wk_d)
                nc.sync.dma_start(bia_sb[:], bia_d)

            def rep_body(first_rep):
              col0 = 0
              for ci, W in enumerate(chunks):
                first = (first_rep and ci == 0)
                hfs = []
                off = 0
                while off < W:
                    hw = min(512, W - off)
                    hfs.append((off, hw))
                    off += hw

                # ---- load z chunk (feature-major, per-k split on chunk 0) ----
                zt_n = zp.tile([P, 4 * W], bf16, tag="zt")
                if "zload" in CFG["ablate"]:
                    nc.gpsimd.memset(zt_n[0:1, 0:2], 0.0)
                elif first:
                    for k in range(4):
                        nc.sync.dma_start(w1_sb[:, k * 512:(k + 1) * 512],
                                          w1v[:, k, :])
                        nc.sync.dma_start(
                            zt_n[:, k * W:(k + 1) * W],
                            zt_d[k * P:(k + 1) * P, col0:col0 + W])
                    nc.sync.dma_start(wk_sb[:], wk_d)
                    nc.sync.dma_start(bia_sb[:], bia_d)
                else:
                    nc.sync.dma_start(
                        zt_n[:].rearrange("p (k c) -> p k c", k=4),
                        zt_d[:, col0:col0 + W]
                            .rearrange("(k p) c -> p k c", p=P))

                # helper: write h = lrelu(psum + bias) into dst.
                # mode "act": one ACT op (Prelu with bias).
                # mode "dve": DVE adds bias PSUM -> SBUF f32 scratch, then
                #             one DVE op lrelu(v) = max(0.2*v, v). (GPSIMD
                #             cannot read PSUM nor run TensorScalarPtr/STT.)
                def act_or_dve(dst, pst_v, b_ap, mode, W=W):
                    if "acts" in CFG["ablate"]:
                        return
                    if not CFG["dve_acts"]:
                        mode = "act"
                    if mode == "act":
                        nc.scalar.activation(dst, pst_v, AF.Prelu,
                                             bias=b_ap, alpha=0.2)
                    else:
                        sc = scp.tile([P, W], f32, tag="sc")
                        nc.vector.tensor_scalar(sc[:], pst_v, b_ap, None,
                                                Alu.add)
                        nc.vector.scalar_tensor_tensor(
                            dst, sc[:], 0.2, sc[:], Alu.mult, Alu.max)

                abl = CFG["ablate"]
                HB = min(W, 1024)       # psum tile width (<= 2 banks)
                NH = W // HB            # halves per m-tile
                # ---- L1: 512 -> 512 ----
                h1_n = zt_n if "acts" in abl else hp.tile(
                    [P, 4 * W], bf16, tag="h1")
                for m in range(4):
                    psts = [big.tile([P, HB], f32, tag="big",
                                     name=f"pst{h}") for h in range(NH)]
                    for k in range(4):
                        for h in range(NH):
                            for o2 in range(0, HB, 512):
                                nc.tensor.matmul(
                                    psts[h][:, o2:o2 + 512],
                                    w1k(k)[:, m * 128:(m + 1) * 128],
                                    zt_n[:, k * W + h * HB + o2:
                                         k * W + h * HB + o2 + 512],
                                    start=(k == 0), stop=(k == 3))
                    for h in range(NH):
                        act_or_dve(
                            h1_n[:, m * W + h * HB:m * W + (h + 1) * HB],
                            psts[h][:], bia(m), "act")

                # ---- L2: 512 -> 256 ----
                h2_n = zt_n if "acts" in abl else hp.tile(
                    [P, 2 * W], bf16, tag="h2")
                for m in range(2):
                    psts = [big.tile([P, HB], f32, tag="big",
                                     name=f"pst{h}") for h in range(NH)]
                    for k in range(4):
                        for h in range(NH):
                            for o2 in range(0, HB, 512):
                                nc.tensor.matmul(
                                    psts[h][:, o2:o2 + 512],
                                    w2v[:, k * 256 + m * 128:
                                        k * 256 + (m + 1) * 128],
                                    h1_n[:, k * W + h * HB + o2:
                                         k * W + h * HB + o2 + 512],
                                    start=(k == 0), stop=(k == 3))
                    for h in range(NH):
                        act_or_dve(
                            h2_n[:, m * W + h * HB:m * W + (h + 1) * HB],
                            psts[h][:], bia(4 + m), "act")

                # ---- L3: 256 -> 128 ----
                h3_n = zt_n if "acts" in abl else hp.tile(
                    [P, W], bf16, tag="h3")
                psts = [big.tile([P, HB], f32, tag="big",
                                 name=f"pst{h}") for h in range(NH)]
                for k in range(2):
                    for h in range(NH):
                        for o2 in range(0, HB, 512):
                            nc.tensor.matmul(
                                psts[h][:, o2:o2 + 512],
                                w3v[:, k * 128:(k + 1) * 128],
                                h2_n[:, k * W + h * HB + o2:
                                     k * W + h * HB + o2 + 512],
                                start=(k == 0), stop=(k == 1))
                if "acts" not in CFG["ablate"]:
                    for h in range(NH):
                        nc.scalar.activation(
                            h3_n[:, h * HB:(h + 1) * HB], psts[h][:],
                            AF.Prelu, bias=bia(6), alpha=0.2)

                # ---- L4: 128 -> 64 (plus ones row for L5 bias) ----
                h4_n = zt_n if "acts" in abl else hp.tile(
                    [65, W], bf16, tag="h4")
                psts = [big.tile([P, HB], f32, tag="big",
                                 name=f"pst{h}") for h in range(NH)]
                for h in range(NH):
                    for o2 in range(0, HB, 512):
                        nc.tensor.matmul(
                            psts[h][0:64, o2:o2 + 512], w4v[:],
                            h3_n[:, h * HB + o2:h * HB + o2 + 512],
                            start=True, stop=True)
                if "acts" not in CFG["ablate"]:
                    for h in range(NH):
                        nc.scalar.activation(
                            h4_n[0:64, h * HB:(h + 1) * HB],
                            psts[h][0:64, :], AF.Prelu,
                            bias=bia(7)[0:64], alpha=0.2)
                    nc.sync.dma_start(h4_n[64:65, :], ones_sb[0:1, 0:W])

                # ---- L5 (feature-major) + QP + store ----
                # W5 columns are host-permuted to [g][c2][o] order and W5 is
                # the stationary operand (2 LDWs instead of 16): the two
                # m-tiles land as p_g0 = position params (x,y) and
                # p_g1 = size params (w,h), partition-aligned so the QP is
                # plain full-tile elementwise work on DVE. Output is stored
                # transposed, (256, BSC); the host untransposes.
                fq = f16 if CFG["qp_f16"] else f32
                p_g0 = p_g1 = None
                if "l5copy" not in abl or "qp" not in abl:
                    p_g0 = stg.tile([P, W], fq, tag="pg0")
                    p_g1 = stg.tile([P, W], fq, tag="pg1")
                for m in range(2):
                    dstt = p_g0 if m == 0 else p_g1
                    for q in range(W // 512):
                        p5 = ps5p.tile([P, 512], f32, tag="l5")
                        nc.tensor.matmul(
                            p5[:], w5v[0:65, m * 128:(m + 1) * 128],
                            h4_n[0:65, q * 512:(q + 1) * 512],
                            start=True, stop=True)
                        if "l5copy" not in abl:
                            nc.vector.tensor_copy(
                                dstt[:, q * 512:(q + 1) * 512], p5[:])

                if b0 == b1 and b2 == b3:
                    groups = [(slice(0, 128), lo_x, hi_x)]
                else:
                    groups = [(slice(0, 64), lo_x, hi_x),
                              (slice(64, 128), lo_y, hi_y)]
                if "qp" in abl:
                    groups = []
                ox = ow = None
                if "qp" not in abl:
                    ox = stg.tile([P, W], f16, tag="ox")
                    ow = stg.tile([P, W], f16, tag="ow")
                for psl, lo, hi in groups:
                    pp = p_g0[psl, :]
                    pg = p_g1[psl, :]
                    g0 = tmp.tile([P, W], fq, tag="g0")
                    s0 = tmp.tile([P, W], fq, tag="s0")
                    u = tmp.tile([P, W], fq, tag="u")
                    t1 = tmp.tile([P, W], fq, tag="t1")
                    # g0 = max(pg, 1); s0 = max(pp, lo) + g0; w = min(s0, hi)
                    nc.vector.tensor_scalar_max(g0[psl, :], pg, 1.0)
                    nc.vector.scalar_tensor_tensor(
                        s0[psl, :], pp, lo, g0[psl, :], Alu.max, Alu.add)
                    nc.vector.tensor_scalar_min(ow[psl, :], s0[psl, :], hi)
                    # u = (pp + hi) - pg ; scale+clip
                    nc.vector.scalar_tensor_tensor(
                        u[psl, :], pp, hi, pg, Alu.add, Alu.subtract)
                    nc.vector.tensor_scalar(u[psl, :], u[psl, :], 0.5,
                                            hi - 1.0, Alu.mult, Alu.min)
                    # x = max(min(max(pp, lo), u), lo)
                    nc.vector.scalar_tensor_tensor(
                        t1[psl, :], pp, lo, u[psl, :], Alu.max, Alu.min)
                    nc.vector.tensor_scalar_max(ox[psl, :], t1[psl, :], lo)

                if "store" not in abl:
                    sx = ox if "qp" not in abl else p_g0
                    sw = ow if "qp" not in abl else p_g1
                    nc.sync.dma_start(o_d[0:128, col0:col0 + W], sx[:])
                    nc.sync.dma_start(o_d[128:256, col0:col0 + W], sw[:])
                col0 += W

            if hw_loop:
                # timing mode: weights loaded once, then a device-side loop
                # of `hw_loop` iterations, each running `reps` rep bodies.
                load_weights()
                rep_body(False)
                with tc.For_i(0, hw_loop, 1):
                    for _ in range(reps):
                        rep_body(False)
            else:
                for rep in range(reps):
                    rep_body(rep == 0)

    nc.compile()
    return nc


def _get_nc(b0, b1, b2, b3, reps=1, chunks=(2048,), hw_loop=0):
    key = (b0, b1, b2, b3, reps, tuple(chunks), hw_loop)
    if key not in _cache:
        _cache[key] = _build(b0, b1, b2, b3, reps, chunks, hw_loop)
    return _cache[key]


def _prep_inputs(z, bounds, W1, c1, W2, c2, W3, c3, W4, c4, W5, c5):
    import ml_dtypes
    bft = ml_dtypes.bfloat16

    b = np.asarray(bounds, np.float32)
    W1m = np.ascontiguousarray(W1[:Z], np.float32).astype(bft)
    b1 = (np.asarray(c1, np.float32)
          + b @ np.asarray(W1[Z:], np.float32)).astype(np.float32)

    wk = np.zeros((P, _WKW), bft)
    wk[:, _W2O:_W2O + 1024] = (np.asarray(W2, np.float32)
                               .reshape(4, P, 256).transpose(1, 0, 2)
                               .reshape(P, 1024).astype(bft))
    wk[:, _W3O:_W3O + 256] = (np.asarray(W3, np.float32)
                              .reshape(2, P, 128).transpose(1, 0, 2)
                              .reshape(P, 256).astype(bft))
    wk[:, _W4O:_W4O + 64] = np.asarray(W4, np.float32).astype(bft)
    w5a = np.concatenate(
        [np.asarray(W5, np.float32), np.asarray(c5, np.float32)[None, :]], 0)
    qidx = np.arange(256)
    gq, c2q, oq = qidx // 128, (qidx // 64) % 2, qidx % 64
    w5a = w5a[:, oq * 4 + 2 * gq + c2q]
    wk[0:65, _W5O:_W5O + 256] = w5a.astype(bft)

    bia = np.zeros((P, 8), np.float32)
    bia[:, 0:4] = b1.reshape(4, P).T
    bia[:, 4:6] = np.asarray(c2, np.float32).reshape(2, P).T
    bia[:, 6] = np.asarray(c3, np.float32)
    bia[0:64, 7] = np.asarray(c4, np.float32)

    zT = np.ascontiguousarray(np.asarray(z, np.float32).T).astype(bft)
    common = {"w1": W1m, "wk": wk, "bia": bia}
    in_maps = []
    for i in range(NCORES):
        m = dict(common)
        m["zt"] = np.ascontiguousarray(zT[:, i * BSC:(i + 1) * BSC])
        in_maps.append(m)
    return in_maps, (float(b[0]), float(b[1]), float(b[2]), float(b[3]))


def kernel(z, bounds, W1, c1, W2, c2, W3, c3, W4, c4, W5, c5):
    from concourse.bass_utils import run_bass_kernel_spmd

    in_maps, bvals = _prep_inputs(z, bounds, W1, c1, W2, c2, W3, c3,
                                  W4, c4, W5, c5)
    nc = _get_nc(*bvals)
    res = run_bass_kernel_spmd(nc, in_maps, core_ids=list(range(NCORES)))
    out = np.concatenate(
        [np.asarray(r["o"], np.float32).T for r in res.results], axis=0)
    return (out.reshape(BS, 2, 2, NOBJ).transpose(0, 3, 1, 2)
            .reshape(BS, NOBJ, 4))
